# revision 28
# baseline (speedup 1.0000x reference)
"""Trainium2 Bass kernel for AnchorProcessor (nms_detection).

Input  x: [8, 255, 128, 128] f32.  Output: [8, 18, 128, 128] f32.
Strategy: shard along H across 8 cores (16 rows each); per-core problem is
fully local (the buggy cross-batch max/argmax reduces over (N, cls), both
on-core), so no collectives.

v2 architecture (v1 in kernel_v1_backup.py ran 168us; DMA floor is ~80us):
  - Full-channel PE transposes: per (n, h) two transposes move channels
    [4:132) and [132:255) of x[n, :, h, :] into one PSUM tile [128pix, 252],
    so obj + all 3 anchors' logits arrive pixel-major together (256 big
    matmuls total vs 406 small ones in v1, and no separate obj transposes).
  - Input DMA as 16 big n-major tiles [128ch, 16h, 128w] with 8KB
    descriptors; compute pipelines per (n, h-quad) chunk behind the DMA.
  - negscore = (lg * -1) * obj is ONE scalar_tensor_tensor per chunk (obj
    columns broadcast along c) instead of v1's ACT/DVE per-(n,a) splits.
  - Work spread across engines (DVE was 116us in v1, gpsimd idle):
      mul: DVE n0-4, Pool n5, ACT n6-7 (per-partition obj scale)
      pack (bit-pack value|index): Pool (gpsimd)
      c-reduce (min over 80 classes, axis=X): DVE only (Pool can't do X)
      cross-n min + unpack: DVE (tiny)
      box path: ACT sigmoid/scale + Pool grid adds in a [96, 512] layout.
  - Same bit-pack argmax trick as v1: packed = (negscore&~0x3FF) | idx via
    (or,xor) stt; one min-reduce gives smax (2^-13 rel quant) + argmax.
"""

import os
import sys

for _p in ("/opt/trn_rl_repo", "/root/.axon_site/_ro/trn_rl_repo"):
    if _p not in sys.path:
        sys.path.append(_p)

import numpy as np

from concourse import bacc, masks, mybir
from concourse.tile import TileContext

N = 8          # batch
A = 3          # anchors
CLS = 80       # classes per anchor
W = 128        # width
HL = 16        # local H rows per core (128 / 8 cores)
NCORES = 8

ANCHOR_W = (116.0, 156.0, 373.0)
ANCHOR_H = (90.0, 198.0, 326.0)

F32 = mybir.dt.float32
U32 = mybir.dt.uint32

# engine assignment knobs (tunable via env for experiments)
def _env_tuple(name, default):
    v = os.environ.get(name)
    if v is None:
        return default
    return tuple(int(s) for s in v.split(",") if s != "")


MUL_DVE_NS = _env_tuple("MUL_DVE", ())   # mul on DVE (from SBUF copy)
MUL_ACT_NS = _env_tuple("MUL_ACT", ())   # mul fused on ACT (from PSUM)


def build_nc(hl=HL, reps=1, mul_dve=MUL_DVE_NS, mul_act=MUL_ACT_NS):
    import contextlib

    hqs = 4 if hl % 4 == 0 else hl     # h rows per chunk
    nch = hl // hqs                    # chunks per n
    bf = hl * W                        # box free size
    bp = A * N                         # box partitions

    nc = bacc.Bacc("TRN2", target_bir_lowering=False, debug=False)

    x = nc.declare_dram_parameter("x", [N, 255, hl, W], F32, isOutput=False)
    grid = nc.declare_dram_parameter("grid", [2, bp, bf], F32, isOutput=False)
    anch = nc.declare_dram_parameter("anch", [2, bp, 1], F32, isOutput=False)
    iota = nc.declare_dram_parameter("iota", [N * CLS], U32, isOutput=False)
    bits = nc.declare_dram_parameter("bits", [4], U32, isOutput=False)
    out = nc.declare_dram_parameter("out", [N, A * 6, hl, W], F32, isOutput=True)
    oscr = nc.dram_tensor("oscratch", [A * 2, hl, W], F32)

    with TileContext(nc) as tc:
        with (
            tc.tile_pool(name="const", bufs=1) as constp,
            tc.tile_pool(name="xt", bufs=3) as xp,
            tc.tile_pool(name="box", bufs=1) as boxp,
            tc.tile_pool(name="neg", bufs=3) as negp,
            tc.tile_pool(name="pak", bufs=3) as pakp,
            tc.tile_pool(name="red", bufs=1) as redp,
            tc.tile_pool(name="outsb", bufs=6) as outsbp,
            tc.tile_pool(name="ps", bufs=3, space="PSUM") as psp,
            tc.tile_pool(name="ps2", bufs=2, space="PSUM") as ps2p,
        ):
            ident = constp.tile([128, 128], F32)
            masks.make_identity(nc, ident[:, :])
            neg1 = constp.tile([128, 1], F32)
            nc.gpsimd.memset(neg1[:, :], -1.0)

            bitst = constp.tile([128, 4], U32)
            nc.sync.dma_start(
                out=bitst[:, :],
                in_=bits[:].unsqueeze(0).broadcast_to([128, 4]),
            )
            iotat = constp.tile([128, N * CLS], U32)
            nc.scalar.dma_start(
                out=iotat[:, :],
                in_=iota[:].unsqueeze(0).broadcast_to([128, N * CLS]),
            )
            gridt = [constp.tile([bp, bf], F32, name=f"grid{g}") for g in range(2)]
            ancht = [constp.tile([bp, 1], F32, name=f"anch{g}") for g in range(2)]
            for g in range(2):
                nc.scalar.dma_start(out=gridt[g][:, :], in_=grid[g, :, :])
                nc.scalar.dma_start(out=ancht[g][:, :], in_=anch[g, :, :])
            # (scalar-queue DMAs above are small; all bulk loads go on the
            # sync queue, whose descriptors spread across all 16 DMA engines)

            loop_cm = (
                tc.For_i(0, reps, 1, hint_engines=(mybir.EngineType.PE,))
                if reps > 1 else contextlib.nullcontext()
            )
            with loop_cm:
                body(nc, tc, x, out, oscr, hl, hqs, nch, bf, bp,
                     ident, bitst, iotat, gridt, ancht, neg1,
                     mul_dve, mul_act,
                     xp, boxp, negp, pakp, redp, outsbp, psp, ps2p)

    nc.compile()
    return nc


def body(nc, tc, x, out, oscr, hl, hqs, nch, bf, bp,
         ident, bitst, iotat, gridt, ancht, neg1,
         mul_dve, mul_act,
         xp, boxp, negp, pakp, redp, outsbp, psp, ps2p):
    # ---------------- input DMA: n-major, 8KB descriptors ----------------
    xt = []
    for n in range(N):
        t0 = xp.tile([128, hl, W], F32, tag="xb0", name=f"xt{n}b0")
        nc.sync.dma_start(out=t0[:, :, :], in_=x[n, 4:132, :, :])
        # both streams on the sync queue (the scalar HW queue serializes
        # narrow-address-span bulk onto 3-4 DMA engines). A 16-descriptor
        # canary rewrite of real tile bytes follows each load: per-engine
        # FIFOs mean the canary lands after every packet of the load, and
        # the tile framework orders consumers after the canary, closing
        # the cross-instruction completion-count race.
        t1 = xp.tile([128, hl, W], F32, tag="xb1", name=f"xt{n}b1")
        nc.sync.dma_start(out=t1[0:123, :, :], in_=x[n, 132:255, :, :])
        nc.sync.dma_start(
            out=t0[0:16, 0, 0:1], in_=x[n, 4:20, 0:1, 0:1])
        nc.sync.dma_start(
            out=t1[0:16, 0, 0:1], in_=x[n, 132:148, 0:1, 0:1])
        xt.append((t0, t1))

    # box input DMA (12 channels re-read in an ACT-friendly layout)
    boxin = []
    for k in range(4):
        bt = boxp.tile([bp, bf], F32, tag=f"bi{k}", name=f"bi{k}")
        eng = nc.sync
        eng.dma_start(
            out=bt[:, :],
            in_=x[:, k:255:85, :, :].transpose([1, 0, 2, 3]),
        )
        boxin.append(bt)

    # per-n c-reduced partial results (all reduces on DVE)
    red = redp.tile([128, N, hl, A], F32, name="red")

    # ---------------- score path ----------------
    # pack/reduce run at PAIR granularity (2 chunks) to amortize DVE op
    # overhead; mul/evict stay per-chunk for pipelining.
    pair = 2 if nch % 2 == 0 else 1
    for n in range(N):
        negsc = None
        for c in range(nch):
            tps = psp.tile([128, hqs, 256], F32, tag="tps")
            for j in range(hqs):
                h = c * hqs + j
                nc.tensor.transpose(
                    tps[:, j, 0:128], xt[n][0][:, h, :], ident[:, :])
                nc.tensor.transpose(
                    tps[:, j, 128:251], xt[n][1][0:123, h, :],
                    ident[:123, :123])

            if c % pair == 0:
                negsc = negp.tile([128, pair, hqs, A, CLS], F32, tag="negsc")
            ci = c % pair
            # negscore = (lg * -1) * obj  [128pix, hqs, A, CLS]
            nob = negp.tile([128, hqs, A], F32, tag="nob")
            nc.vector.tensor_scalar_mul(
                nob[:, :, :], tps[:, :, 0:171:85], -1.0)
            if n in mul_act:
                # fused path: ACT reads PSUM, per-(h, a) per-partition scale
                for j in range(hqs):
                    for a in range(A):
                        nc.scalar.mul(
                            negsc[:, ci, j, a, :],
                            tps[:, j, a * 85 + 1:a * 85 + 81],
                            nob[:, j, a:a + 1],
                        )
            elif n in mul_dve:
                # DVE multiplies straight from PSUM (one op per anchor)
                for a in range(A):
                    nc.vector.tensor_tensor(
                        out=negsc[:, ci, :, a, :],
                        in0=tps[:, :, a * 85 + 1:a * 85 + 81],
                        in1=nob[:, :, a:a + 1].broadcast_to([128, hqs, CLS]),
                        op=mybir.AluOpType.mult,
                    )
            else:
                # ACT evicts PSUM -> SBUF in one big copy; Pool multiplies
                # from SBUF (Pool cannot access PSUM)
                tsb = negp.tile([128, hqs, 251], F32, tag="tsb")
                nc.scalar.copy(tsb[:, :, :], tps[:, :, 0:251])
                for a in range(A):
                    nc.gpsimd.tensor_tensor(
                        out=negsc[:, ci, :, a, :],
                        in0=tsb[:, :, a * 85 + 1:a * 85 + 81],
                        in1=nob[:, :, a:a + 1].broadcast_to([128, hqs, CLS]),
                        op=mybir.AluOpType.mult,
                    )

            if c % pair != pair - 1:
                continue
            # pack = (negscore & ~0x3FF) | idx  via (|0x3FF) ^ (idx^0x3FF)
            # (bitwise ops are DVE-only; Pool rejects them)
            packed = pakp.tile([128, pair, hqs, A, CLS], F32, tag="packed")
            iota_ap = iotat[:, n * CLS:(n + 1) * CLS].unsqueeze(1).broadcast_to(
                [128, pair * hqs * A, CLS])
            nc.vector.scalar_tensor_tensor(
                out=packed[:, :, :, :, :].rearrange(
                    "p x h a c -> p (x h a) c").bitcast(U32),
                in0=negsc[:, :, :, :, :].rearrange(
                    "p x h a c -> p (x h a) c").bitcast(U32),
                scalar=bitst[:, 0:1],
                in1=iota_ap,
                op0=mybir.AluOpType.bitwise_or,
                op1=mybir.AluOpType.bitwise_xor,
            )

            # min over classes (axis=X) -> [128, pair*hqs, A]
            c0 = c - pair + 1
            nc.vector.tensor_reduce(
                red[:, n, c0 * hqs:(c + 1) * hqs, :],
                packed[:, :, :, :, :].rearrange("p x h a c -> p (x h) a c"),
                axis=mybir.AxisListType.X,
                op=mybir.AluOpType.min,
            )

        # ---- interleave box compute after n==1's chunks are issued ----
        if n == 1:
            for k in range(4):
                bo = boxp.tile([bp, bf], F32, tag=f"bo{k}", name=f"bo{k}")
                if k < 2:
                    nc.scalar.activation(
                        bo[:, :], boxin[k][:, :],
                        mybir.ActivationFunctionType.Sigmoid)
                    nc.gpsimd.tensor_add(bo[:, :], bo[:, :], gridt[k][:, :])
                else:
                    nc.scalar.mul(bo[:, :], boxin[k][:, :], ancht[k - 2][:, :])
                nc.sync.dma_start(
                    out=out[:, k:18:6, :, :].transpose([1, 0, 2, 3]),
                    in_=bo[:, :],
                )

    # ---------------- cross-n min + unpack + output ----------------
    m = redp.tile([128, hl, A], F32, name="m")
    nc.vector.tensor_reduce(
        m[:, :, :],
        red[:, :, :, :].rearrange("p n h a -> p h a n"),
        axis=mybir.AxisListType.X,
        op=mybir.AluOpType.min,
    )

    # vq = packed & ~0x3FF (negated smax, quantized); sarg = low 10 bits
    vq = redp.tile([128, hl * A], F32, name="vq")
    nc.vector.scalar_tensor_tensor(
        out=vq[:, :].bitcast(U32),
        in0=m[:, :, :].rearrange("p h a -> p (h a)").bitcast(U32),
        scalar=bitst[:, 0:1],
        in1=bitst[:, 0:1].broadcast_to([128, hl * A]),
        op0=mybir.AluOpType.bitwise_or, op1=mybir.AluOpType.bitwise_xor,
    )
    sargT = redp.tile([128, hl * A], F32, name="sargT")
    nc.vector.scalar_tensor_tensor(
        out=sargT[:, :].bitcast(U32),
        in0=m[:, :, :].rearrange("p h a -> p (h a)").bitcast(U32),
        scalar=bitst[:, 0:1],
        in1=bitst[:, 1:2].broadcast_to([128, hl * A]),
        op0=mybir.AluOpType.bitwise_and, op1=mybir.AluOpType.bitwise_or,
    )
    nc.vector.scalar_tensor_tensor(
        out=sargT[:, :], in0=sargT[:, :], scalar=1.0,
        in1=bitst[:, 2:3].bitcast(F32).broadcast_to([128, hl * A]),
        op0=mybir.AluOpType.subtract, op1=mybir.AluOpType.mult,
    )

    for a in range(A):
        for t_in, ch_out, scl in ((vq, a * 6 + 4, -1.0),
                                  (sargT, a * 6 + 5, 1.0)):
            t3 = t_in[:, :].rearrange("p (h a) -> p h a", a=A)[:, :, a]
            tpo = ps2p.tile([hl, 128], F32, tag="outps")
            nc.tensor.transpose(tpo[:, :], t3, ident[:, :])
            osb = outsbp.tile([hl, 128], F32, tag="osb")
            if scl == 1.0:
                nc.scalar.copy(osb[:, :], tpo[:, :])
            else:
                nc.scalar.mul(osb[:, :], tpo[:, :], scl)
            si = (ch_out % 6 - 4) * A + a
            nc.sync.dma_start(out=oscr[si, :, :], in_=osb[:, :])
            nc.sync.dma_start(
                out=out[:, ch_out, :, :],
                in_=oscr[si, :, :].unsqueeze(0).broadcast_to([N, hl, W]),
            )


_NC_CACHE = {}


def get_nc(hl=HL):
    if hl not in _NC_CACHE:
        _NC_CACHE[hl] = build_nc(hl)
    return _NC_CACHE[hl]


def make_in_maps(x, hl=HL):
    """Shard the full input along H and build per-core input maps."""
    x = np.ascontiguousarray(x, dtype=np.float32)
    bf = hl * W
    bp = A * N

    gx = np.tile(np.arange(W, dtype=np.float32), hl)             # value = w
    anch_col = np.stack(
        [np.repeat(np.array(ANCHOR_W, np.float32), N),
         np.repeat(np.array(ANCHOR_H, np.float32), N)]
    ).reshape(2, bp, 1)
    iota_bits = np.arange(N * CLS, dtype=np.uint32) ^ 0x3FF
    bits = np.array([0x3FF, 0x3F800000, 0x4B000000, 0],
                    np.uint32)  # masklo, bits(1.0), bits(2^23), unused
    in_maps = []
    ncores = x.shape[2] // hl
    for i in range(ncores):
        grid = np.empty((2, bp, bf), np.float32)
        grid[0] = gx
        gy = np.repeat(np.arange(i * hl, (i + 1) * hl, dtype=np.float32), W)
        grid[1] = gy
        in_maps.append({
            "x": np.ascontiguousarray(x[:, :, i * hl:(i + 1) * hl, :]),
            "grid": grid,
            "anch": anch_col,
            "iota": iota_bits,
            "bits": bits,
        })
    return in_maps


def patch_compile_cache(cache_dir="/tmp/bass_neff_cache"):
    """Cache compiled NEFFs on disk keyed by the BIR hash (compile takes
    minutes; the cache makes repeated runs of an identical graph instant)."""
    import hashlib
    import shutil
    import concourse.bass2jax as b2j

    if getattr(b2j, "_neff_cache_patched", False):
        return
    os.makedirs(cache_dir, exist_ok=True)
    orig = b2j.compile_bir_kernel

    def cached(bir_json, tmpdir, neff_name="file.neff"):
        data = bir_json if isinstance(bir_json, bytes) else str(bir_json).encode()
        key = hashlib.sha256(data).hexdigest()[:32]
        cpath = os.path.join(cache_dir, key + ".neff")
        if os.path.exists(cpath):
            opath = os.path.join(tmpdir, neff_name)
            shutil.copy(cpath, opath)
            return opath
        r = orig(bir_json, tmpdir, neff_name)
        try:
            shutil.copy(r, cpath)
        except OSError:
            pass
        return r

    b2j.compile_bir_kernel = cached
    b2j._neff_cache_patched = True


def kernel(x: np.ndarray) -> np.ndarray:
    from concourse.bass_utils import run_bass_kernel_spmd

    patch_compile_cache()

    nc = get_nc(HL)
    in_maps = make_in_maps(x, HL)
    res = run_bass_kernel_spmd(nc, in_maps, core_ids=list(range(NCORES)))
    return np.concatenate([res.results[i]["out"] for i in range(NCORES)], axis=2)
